# revision 10
# baseline (speedup 1.0000x reference)
"""Muskingum-Cunge river routing (depth-13 binary tree, N=8191, T=2048) on
8 Trainium2 NeuronCores — parallel-in-time Picard solver, v2.

Per reach, the MC update O_t = C1 I_t + C2 I_{t-1} + C3 O_{t-1} is a linear
recurrence once the (flow-dependent) coefficients are frozen; each Picard
pass recomputes the coefficients from the previous trajectory and re-solves
the recurrence with one DVE/Pool tensor_tensor_scan along time. The relu
clamp is folded into the fixed point by scanning the unclamped accumulator U
and adding C3*relu(-U_prev) to the next pass's RHS.

v2 structure (the scan instruction is serial along the free dim — ~9 cycles
per column regardless of partition count — so total scan instances are the
budget that matters):
  - levels 12..10 (512/256/128 local nodes): node-major [128, T] tiles,
    K_BIG=2 Picard passes each, exact child inputs (7 tiles, 14 scans).
  - levels 9..3 (127 local nodes): ONE stacked [127, T] system solved with
    S_STACK=8 joint Jacobi-Picard passes; the parent<-children pair-sum is a
    precomputed 0/1 matmul on the otherwise idle PE (A.T stationary), so a
    pass is ~25 fat instructions and there is no chunked layout, no DRAM
    scatter/gather, and no cross-partition carry machinery.
  - AllGather of the 8 level-3 root trajectories (64 KB), then the 7-node
    top tree (levels 2..0) is another stacked system, S_TOP passes,
    replicated on every core.
Scans alternate between DVE and Pool so independent solves overlap.

Sharding: core c owns the subtree rooted at the c-th level-3 node (1023
reaches); only the 8 level-3 root trajectories are all-gathered.
"""
import os
import sys

import numpy as np

for _p in ("/opt/trn_rl_repo", "/root/.axon_site/_ro/trn_rl_repo"):
    if os.path.isdir(_p) and _p not in sys.path:
        sys.path.insert(0, _p)

DEPTH = 13
N = 2**DEPTH - 1
T = 2048
NC = 8
F32 = np.float32

K_BIG = 2          # Picard passes per big level (12, 11, 10)
S_STACK = 8        # joint Jacobi-Picard passes for stacked levels 9..3
S_TOP = 4          # joint passes for the top tree (levels 2..0)

BIG_LEVELS = [(12, 512), (11, 256), (10, 128)]
STACK_LEVELS = [9, 8, 7, 6, 5, 4, 3]           # 64+32+16+8+4+2+1 = 127 rows
ALL_LAT_LEVELS = [12, 11, 10, 9, 8, 7, 6, 5, 4, 3, 2, 1, 0]
R_STACK = 127
R_TOP = 7
NBIG_TILES = 4 + 2 + 1
NGRP = NBIG_TILES + 2                          # const col groups: 7 big + stack + top
NCOL = NGRP * 4 + 1
_LN2DT_COL = NGRP * 4
AMAT_COLS = R_STACK + R_TOP                    # stack lhsT (127) + top lhsT (7)


def _build_ord():
    ORD = [np.array([0], dtype=np.int64)]
    for l in range(DEPTH - 1):
        cur = ORD[l]
        nxt = np.empty(2 * len(cur), dtype=np.int64)
        nxt[: len(cur)] = 2 * cur + 1
        nxt[len(cur):] = 2 * cur + 2
        ORD.append(nxt)
    return ORD


ORD = _build_ord()


def _level_nodes(core, lv):
    return ORD[lv] if lv < 3 else ORD[lv][core::NC]


def _lat_row(lv):
    off = 0
    for l in ALL_LAT_LEVELS:
        if l == lv:
            return off
        off += len(ORD[l]) // (NC if l >= 3 else 1)
    raise KeyError(lv)


LAT_ROWS = sum(len(ORD[l]) // (NC if l >= 3 else 1) for l in ALL_LAT_LEVELS)

# stack row base per level (level 9 first)
_STACK_BASE = {}
_off = 0
for _lv in STACK_LEVELS:
    _STACK_BASE[_lv] = _off
    _off += len(ORD[_lv]) // NC


def _stack_amat_T():
    """[127, 127] lhsT: A_T[child_row, parent_row] = 1 (in-stack coupling)."""
    A = np.zeros((R_STACK, R_STACK), F32)
    for lv in STACK_LEVELS[1:]:                 # parents: levels 8..3
        nl = len(ORD[lv]) // NC
        bp = _STACK_BASE[lv]
        bc = _STACK_BASE[lv + 1]
        for j in range(nl):
            A[bc + j, bp + j] = 1.0
            A[bc + j + nl, bp + j] = 1.0
    return A


def _top_amat_T():
    """[7, 7] lhsT for top rows: 0..3 = level-2 canon (ORD[2]), 4..5 =
    level-1 canon (ORD[1]), 6 = root."""
    A = np.zeros((R_TOP, R_TOP), F32)
    # level-1 canon j (rows 4,5): children = level-2 canon j and j+2
    A[0, 4] = A[2, 4] = 1.0
    A[1, 5] = A[3, 5] = 1.0
    # root (row 6): children = level-1 canon 0,1
    A[4, 6] = A[5, 6] = 1.0
    return A


def _host_precompute(inputs):
    lat = np.ascontiguousarray(np.asarray(inputs["lateral_inflows"], F32))
    n_ = np.asarray(inputs["manning_n"], F32).astype(np.float64)
    L = np.asarray(inputs["lengths"], F32).astype(np.float64)
    S = np.asarray(inputs["slopes"], F32).astype(np.float64)
    wc = np.asarray(inputs["width_coefs"], F32).astype(np.float64)
    we = np.asarray(inputs["width_exps"], F32).astype(np.float64)
    dc = np.asarray(inputs["depth_coefs"], F32).astype(np.float64)
    de = np.asarray(inputs["depth_exps"], F32).astype(np.float64)
    c0 = (5.0 / 3.0) * dc ** (2.0 / 3.0) * np.sqrt(S) / n_
    a1n = -(2.0 / 3.0) * de
    a3 = 1.0 - we - (2.0 / 3.0) * de
    ln_half = np.log(0.5)
    P4 = np.log(L / c0) + a1n * ln_half
    P3 = np.log(0.5 / (wc * S * L * c0)) + a3 * ln_half
    consts = np.stack([a1n, a3, P4, P3]).astype(F32)      # [4, N]

    amat = np.zeros((128, AMAT_COLS), F32)
    amat[:R_STACK, :R_STACK] = _stack_amat_T()
    amat[:R_TOP, R_STACK:] = _top_amat_T()

    in_maps = []
    for core in range(NC):
        lat_rows = []
        for lv in ALL_LAT_LEVELS:
            nodes = _level_nodes(core, lv)
            lat_rows.append(lat[:, nodes].T)
        LAT = np.ascontiguousarray(np.concatenate(lat_rows, axis=0))

        ccols = []
        for lv, n in BIG_LEVELS:
            nodes = _level_nodes(core, lv)
            for ti in range(n // 128):
                ccols.append(consts[:, nodes[ti * 128:(ti + 1) * 128]])
        stack_nodes = np.concatenate(
            [_level_nodes(core, lv) for lv in STACK_LEVELS])
        cs = np.zeros((4, 128), F32)
        cs[:, :R_STACK] = consts[:, stack_nodes]
        ccols.append(cs)
        top_nodes = np.concatenate([ORD[2], ORD[1], ORD[0]])
        ct = np.zeros((4, 128), F32)
        ct[:, :R_TOP] = consts[:, top_nodes]
        ccols.append(ct)
        ccols.append(np.full((1, 128), np.log(2.0 * float(inputs["dt"])), F32))
        CST = np.ascontiguousarray(np.concatenate(ccols, axis=0).T.astype(F32))
        in_maps.append({"lat": LAT, "cst": CST, "amat": amat})
    return in_maps


def _build_bass(dtf, single=False):
    from contextlib import ExitStack

    import concourse.bass as bass
    import concourse.tile as tile
    from concourse import bacc, mybir

    f32 = mybir.dt.float32
    OP = mybir.AluOpType
    AF = mybir.ActivationFunctionType
    inv_dt = 1.0 / dtf

    nc = bacc.Bacc("TRN2", target_bir_lowering=False, debug=False,
                   num_devices=NC)
    lat_d = nc.dram_tensor("lat", [LAT_ROWS, T], f32, kind="ExternalInput").ap()
    cst_d = nc.dram_tensor("cst", [128, NCOL], f32, kind="ExternalInput").ap()
    amat_d = nc.dram_tensor("amat", [128, AMAT_COLS], f32,
                            kind="ExternalInput").ap()
    out_d = nc.dram_tensor("out", [1, T], f32, kind="ExternalOutput").ap()

    scan_ctr = [0]

    with tile.TileContext(nc) as tc, ExitStack() as ctx:
        cpool = ctx.enter_context(tc.tile_pool(name="const", bufs=1))
        opool = ctx.enter_context(tc.tile_pool(name="lvlO", bufs=7))
        spool = ctx.enter_context(tc.tile_pool(name="scr", bufs=1))
        psum = ctx.enter_context(tc.tile_pool(name="ps", bufs=4, space="PSUM"))
        dram = ctx.enter_context(tc.tile_pool(name="dram", bufs=1, space="DRAM"))

        cst = cpool.tile([128, NCOL], f32)
        nc.sync.dma_start(cst[:], cst_d)
        amat = cpool.tile([128, AMAT_COLS], f32)
        nc.sync.dma_start(amat[:], amat_d)

        def cc(grp, R):
            c0 = grp * 4
            return (cst[0:R, c0:c0 + 1], cst[0:R, c0 + 1:c0 + 2],
                    cst[0:R, c0 + 2:c0 + 3], cst[0:R, c0 + 3:c0 + 4])

        def sc(tag, R):
            t = spool.tile([128, T], f32, tag=tag, name=f"scr_{tag}")
            return t[0:R, :]


        def emit_group(R, S, grp, Obuf, Itile, latE=None, lhsT=None):
            """Picard-solve one group of R reaches over the full horizon.

            Itile: [128, T+1] tile; col 0 must be zero. If latE is None the
            inflow in Itile[:,1:] is fixed (big levels). Otherwise each pass
            rebuilds Itile[:,1:] = latE + lhsT.T @ O (stacked groups), and
            Obuf[:,1:] must start zeroed.
            Obuf: [128, T+1]; on return holds O with col 0 = 0.
            """
            a1, a3, P4, P3 = cc(grp, R)
            U = spool.tile([128, T + 1], f32, tag="gU")
            r = spool.tile([128, T + 1], f32, tag="gr")
            nc.vector.memset(r[0:R, 0:1], 0.0)
            nc.vector.memset(Obuf[0:R, 0:1], 0.0)
            I1 = Itile[0:R, 1:]
            I0 = Itile[0:R, 0:T]
            Dfix = None
            if latE is None:
                Dfix = sc("gD", R)
                nc.gpsimd.tensor_sub(Dfix, I1, I0)
            for it in range(S):
                if latE is not None:
                    for j in range(4):
                        ps = psum.tile([128, 512], f32, tag=f"ps{j % 2}")
                        nc.tensor.matmul(ps[0:R, :], lhsT,
                                         Obuf[0:R, 1 + 512 * j:1 + 512 * (j + 1)],
                                         start=True, stop=True)
                        nc.vector.tensor_add(
                            Itile[0:R, 1 + 512 * j:1 + 512 * (j + 1)],
                            ps[0:R, :], latE[0:R, 512 * j:512 * (j + 1)])
                Oold = I0 if it == 0 else Obuf[0:R, 0:T]
                s1 = sc("s1", R)
                nc.gpsimd.tensor_add(s1, I1, Oold)
                s2 = sc("s2", R)
                nc.gpsimd.tensor_scalar(s2, s1, 2e-3, None, op0=OP.max)
                lq = sc("s1", R)
                nc.scalar.activation(lq, s2, AF.Ln)
                K_ = sc("s3", R)
                nc.scalar.activation(K_, lq, AF.Exp, bias=P4, scale=a1)
                tt = sc("s2", R)
                nc.scalar.activation(tt, lq, AF.Exp, bias=P3, scale=a3)
                w1 = sc("s4", R)
                nc.gpsimd.tensor_scalar(w1, tt, 2.0, 1.0, op0=OP.mult,
                                        op1=OP.min)
                v1 = sc("s2", R)
                nc.gpsimd.tensor_mul(v1, K_, w1)
                den = sc("s4", R)
                nc.vector.scalar_tensor_tensor(den, K_, dtf, v1, OP.add,
                                               OP.add)
                lnd = sc("s8", R)
                nc.scalar.activation(lnd, den, AF.Ln)
                q2 = sc("s2", R)
                nc.scalar.activation(q2, lnd, AF.Exp,
                                     bias=cst[0:R, _LN2DT_COL:_LN2DT_COL + 1],
                                     scale=-1.0)
                C3 = sc("s4", R)
                nc.gpsimd.tensor_scalar(C3, q2, -1.0, 1.0, op0=OP.mult,
                                        op1=OP.add)
                p2 = sc("s5", R)
                nc.vector.scalar_tensor_tensor(p2, K_, inv_dt, q2, OP.mult,
                                               OP.mult)
                if Dfix is not None:
                    D = Dfix
                else:
                    D = sc("s3", R)
                    nc.gpsimd.tensor_sub(D, I1, I0)
                z1 = sc("s6", R)
                nc.vector.tensor_mul(z1, p2, D)
                m1 = sc("s5", R)
                nc.gpsimd.tensor_sub(m1, D, z1)
                if it == 0:
                    t2 = sc("s6", R)
                    nc.gpsimd.tensor_mul(t2, q2, I0)
                    B = sc("s1", R)
                    nc.vector.tensor_add(B, m1, t2)
                else:
                    G = sc("s6", R)
                    nc.gpsimd.tensor_sub(G, r[0:R, 0:T], I0)
                    z2 = sc("s7", R)
                    nc.vector.tensor_mul(z2, q2, G)
                    m2 = sc("s6", R)
                    nc.gpsimd.tensor_add(m2, m1, r[0:R, 0:T])
                    B = sc("s1", R)
                    nc.vector.tensor_sub(B, m2, z2)
                nc.vector.tensor_tensor_scan(U[0:R, 1:], C3, B, 0.0,
                                             OP.mult, OP.add)
                nc.scalar.activation(Obuf[0:R, 1:], U[0:R, 1:], AF.Relu)
                if it < S - 1:
                    nc.scalar.activation(r[0:R, 1:], U[0:R, 1:], AF.Relu,
                                         scale=-1.0)

        # ---- big levels (12, 11, 10): node-major tiles, exact inputs ----
        grp = 0
        child_tiles = None
        O10 = None
        for lv, n in BIG_LEVELS:
            ntile = n // 128
            r0 = _lat_row(lv)
            tiles = []
            for ti in range(ntile):
                Ibig = spool.tile([128, T + 1], f32, tag="bI")
                nc.vector.memset(Ibig[:, 0:1], 0.0)
                if lv == 12:
                    nc.sync.dma_start(
                        Ibig[:, 1:], lat_d[r0 + ti * 128:r0 + (ti + 1) * 128, :])
                else:
                    slat = spool.tile([128, T], f32, tag="bL")
                    nc.sync.dma_start(
                        slat[:], lat_d[r0 + ti * 128:r0 + (ti + 1) * 128, :])
                    sup = spool.tile([128, T], f32, tag="bS")
                    nc.vector.tensor_add(sup[:], child_tiles[ti][:, 1:],
                                         child_tiles[ti + ntile][:, 1:])
                    nc.vector.tensor_add(Ibig[:, 1:], slat[:], sup[:])
                Obig = opool.tile([128, T + 1], f32, tag="bigO")
                emit_group(128, K_BIG, grp, Obig, Ibig)
                tiles.append(Obig)
                grp += 1
            child_tiles = tiles
            if lv == 10:
                O10 = tiles[0]

        # ---- stacked levels 9..3: one joint Jacobi-Picard system ----
        latE = spool.tile([128, T], f32, tag="sLat")
        nc.sync.dma_start(latE[0:R_STACK, :],
                          lat_d[_lat_row(9):_lat_row(9) + R_STACK, :])
        tmp = spool.tile([128, T], f32, tag="bS")
        nc.sync.dma_start(tmp[0:64, :], O10[64:128, 1:])
        nc.vector.tensor_add(latE[0:64, :], latE[0:64, :], O10[0:64, 1:])
        nc.vector.tensor_add(latE[0:64, :], latE[0:64, :], tmp[0:64, :])

        Ist = spool.tile([128, T + 1], f32, tag="bI")
        nc.vector.memset(Ist[0:R_STACK, 0:1], 0.0)
        Ost = opool.tile([128, T + 1], f32, tag="bigO")
        nc.gpsimd.memset(Ost[0:R_STACK, 1:], 0.0)
        emit_group(R_STACK, S_STACK, NBIG_TILES, Ost, Ist,
                   latE=latE, lhsT=amat[0:R_STACK, 0:R_STACK])

        # ---- AllGather the 8 level-3 root trajectories ----
        b_in = dram.tile([1, T], f32)
        b_out = dram.tile([NC, T], f32)
        nc.sync.dma_start(b_in[:], Ost[R_STACK - 1:R_STACK, 1:])
        if single:
            zt = spool.tile([8, T], f32, tag="bL")
            nc.vector.memset(zt[:], 0.0)
            nc.sync.dma_start(b_out[1:8, :], zt[0:7, :])
            nc.sync.dma_start(b_out[0:1, :], b_in[:])
        else:
            nc.gpsimd.collective_compute(
                "AllGather", OP.bypass,
                replica_groups=[list(range(NC))],
                ins=[b_in.opt()], outs=[b_out.opt()])

        # ---- top tree (levels 2..0), replicated on every core ----
        latT = spool.tile([128, T], f32, tag="sLat")
        nc.sync.dma_start(latT[0:R_TOP, :],
                          lat_d[_lat_row(2):_lat_row(2) + R_TOP, :])
        rtA = spool.tile([128, T], f32, tag="bS")
        nc.sync.dma_start(rtA[0:4, :], b_out[0:4, :])
        rtB = spool.tile([128, T], f32, tag="bL")
        nc.sync.dma_start(rtB[0:4, :], b_out[4:8, :])
        nc.vector.tensor_add(latT[0:4, :], latT[0:4, :], rtA[0:4, :])
        nc.vector.tensor_add(latT[0:4, :], latT[0:4, :], rtB[0:4, :])

        Itp = spool.tile([128, T + 1], f32, tag="bI")
        nc.vector.memset(Itp[0:R_TOP, 0:1], 0.0)
        Otp = opool.tile([128, T + 1], f32, tag="bigO")
        nc.gpsimd.memset(Otp[0:R_TOP, 1:], 0.0)
        emit_group(R_TOP, S_TOP, NBIG_TILES + 1, Otp, Itp,
                   latE=latT, lhsT=amat[0:R_TOP, R_STACK:])

        nc.sync.dma_start(out_d, Otp[R_TOP - 1:R_TOP, 1:])

    nc.compile()
    return nc


def kernel(**inputs):
    from concourse.bass_utils import run_bass_kernel_spmd

    in_maps = _host_precompute(inputs)
    dtf = float(inputs["dt"])
    nc = _build_bass(dtf)
    res = run_bass_kernel_spmd(nc, in_maps, core_ids=list(range(NC)))
    out = res.results[0]["out"].reshape(-1)
    return out.astype(F32)


if __name__ == "__main__":
    data = np.load("/root/problem/inputs_cache.npz")
    inputs = {k: data[k] for k in data.files}
    out = kernel(**inputs)
    exp = np.load("/root/problem/expected.npy")
    err = np.abs(out - exp) / (np.abs(exp) + 1e-6)
    print("kernel[:4]", out[:4], "expected[:4]", exp[:4])
    print("max rel err", err.max())
